# revision 1
# baseline (speedup 1.0000x reference)
"""GNN message-passing kernel for Trainium2 (Bass/Tile), 8-core SPMD.

Strategy:
- Node tensors live TRANSPOSED on chip: xT [128 features (partitions), nodes].
- Nodes padded 40000 -> 40960 (320 blocks of 128); 8 cores.
- Dense (x@W, BN, relu) is computed replicated on every core (stats are then
  fully local); edge aggregation is sharded: core r owns target nodes
  [r*5120, (r+1)*5120).
- Neighbor aggregation: h = x@Wnb written node-major to a DRAM table; per-edge
  rows are fetched with dma_gather (int16 indices -> two "halves" of the node
  space), and scatter-added with PE one-hot matmuls (fp8 one-hot streamed from
  DRAM, built once on host from edge_index).
- Per-layer aggr shards are exchanged with an AllGather collective.
- Edge branch: ea = segment_sum(edge_attr@Wedge + bedge, row) is factored as
  Wedge (x) s + bedge (x) c_out with s = segment_sum(edge_attr, row) computed
  on device by a row-sorted one-hot matmul pass; degree counts (pure
  edge_index metadata) come from host bincount.  Biases followed by BN (b0,
  bnode, bm1, bm2) cancel inside BN and are dropped; bnb enters via
  c_in (x) bnb added to aggr before its BN.
"""
import numpy as np
import ml_dtypes

F = 128
L = 3
EPS = 1e-5
NREAL = 40000
NE = 640000
NCORES = 8
NP_ = 40960
SHARD = NP_ // NCORES       # 5120
HALF = NP_ // 2             # 20480
NBLK = SHARD // F           # 40 blocks per core
CHUNK = 512                 # dense free-dim chunk
NCHD = NP_ // CHUNK         # 80 dense chunks
GCALL = 1024                # idxs per dma_gather call

BF16 = np.float16
FP8 = ml_dtypes.float8_e4m3


def _ceil(a, b):
    return -(-a // b)


def _wrap_idx16(vals):
    """int16 gather-index layout: value j at [j%16, j//16], tiled to 128 parts."""
    n = vals.shape[0]
    a = vals.reshape(n // 16, 16).T.astype(np.int16)   # [16, n/16]
    return np.tile(a, (8, 1))                          # [128, n/16]


def _prep(node_attr, edge_index, edge_attr):
    """Host-side index preprocessing -> per-core arrays + metadata."""
    row = edge_index[0].astype(np.int64)
    col = edge_index[1].astype(np.int64)
    ea = edge_attr[:, 0].astype(np.float32)

    shard = col // SHARD
    half = row // HALF
    blk = (col % SHARD) // F
    tloc = col % F

    # --- col pass (neighbor aggregation) ---
    cnt = np.zeros((NCORES, 2, NBLK), np.int64)
    np.add.at(cnt, (shard, half, blk), 1)
    sseg = _ceil(max(int(cnt.max()), 1), F) * F
    nch_h = NBLK * sseg // F          # chunks per half
    nstream_h = NBLK * sseg           # slots per half
    # slot within core stream
    order = np.lexsort((blk, half, shard))
    so_shard, so_half, so_blk = shard[order], half[order], blk[order]
    # offset within each (shard, half, blk) group
    grp = ((so_shard * 2 + so_half) * NBLK + so_blk)
    grp_start = np.zeros(NCORES * 2 * NBLK + 1, np.int64)
    np.add.at(grp_start, grp + 1, 1)
    grp_start = np.cumsum(grp_start)
    within = np.arange(NE) - grp_start[grp]
    slot = (so_half * NBLK + so_blk) * sseg + within   # slot in core stream

    gsrc = np.zeros((NCORES, 2 * nstream_h), np.int16)
    gsrc[so_shard, slot] = (row[order] - so_half * HALF).astype(np.int16)
    oh_col = np.zeros((NCORES, 2 * nstream_h, F), np.uint8)
    oh_col[so_shard, slot, tloc[order]] = 1

    gidx = np.stack([_wrap_idx16(gsrc[r]) for r in range(NCORES)])  # [8,128,S/16]
    nch = 2 * nch_h
    # one-hot: [ngrp, 128 part, 8 chunk, 128] fp8, groups of 8 chunks
    ohc = oh_col.reshape(NCORES, nch // 8, 8, F, F).transpose(0, 1, 3, 2, 4)
    ohc = np.ascontiguousarray(ohc).astype(FP8)

    # --- row pass (edge branch s = segment_sum(edge_attr, row)) ---
    rshard = row // SHARD
    rblk = (row % SHARD) // F
    rloc = row % F
    rcnt = np.zeros((NCORES, NBLK), np.int64)
    np.add.at(rcnt, (rshard, rblk), 1)
    rseg = _ceil(max(int(rcnt.max()), 1), F) * F
    nch2 = NBLK * rseg // F
    rorder = np.lexsort((rblk, rshard))
    ro_shard, ro_blk = rshard[rorder], rblk[rorder]
    rgrp = ro_shard * NBLK + ro_blk
    rgs = np.zeros(NCORES * NBLK + 1, np.int64)
    np.add.at(rgs, rgrp + 1, 1)
    rgs = np.cumsum(rgs)
    rwithin = np.arange(NE) - rgs[rgrp]
    rslot = ro_blk * rseg + rwithin

    eav = np.zeros((NCORES, NBLK * rseg), np.float32)
    eav[ro_shard, rslot] = ea[rorder]
    oh_row = np.zeros((NCORES, NBLK * rseg, F), np.uint8)
    oh_row[ro_shard, rslot, rloc[rorder]] = 1
    ohr = oh_row.reshape(NCORES, nch2 // 8, 8, F, F).transpose(0, 1, 3, 2, 4)
    ohr = np.ascontiguousarray(ohr).astype(FP8)
    # eav as [128 (edge within chunk), nch2] for lhsT column slicing
    eav_t = np.ascontiguousarray(
        eav.reshape(NCORES, nch2, F).transpose(0, 2, 1)).astype(BF16)

    # degree counts (pure edge_index metadata)
    c_out = np.bincount(row, minlength=NP_).astype(np.float32)
    c_in = np.bincount(col, minlength=NP_).astype(np.float32)
    ci_pack = np.stack([c_out, c_in])  # [2, NP]

    naT = np.zeros((2, NP_), np.float32)
    naT[:, :NREAL] = node_attr.T
    return dict(sseg=sseg, nch=nch, rseg=rseg, nch2=nch2,
                gidx=gidx, ohc=ohc, ohr=ohr, eav_t=eav_t,
                ci_pack=ci_pack, naT=naT.astype(BF16))


def _build(meta, weights):
    """Build the Bass program. Returns (nc, out_name)."""
    import concourse.bass as bass
    import concourse.tile as tile
    from concourse import bacc, mybir

    sseg, nch, rseg, nch2 = meta["sseg"], meta["nch"], meta["rseg"], meta["nch2"]
    nch_h = nch // 2
    nstream_h = nch_h * F
    ncalls_h = nstream_h // GCALL
    cpb = sseg // F            # chunks per (half, block)
    cpb2 = rseg // F
    dt = mybir.dt
    AX = mybir.AxisListType.X
    OP = mybir.AluOpType
    ACTF = mybir.ActivationFunctionType

    nc = bacc.Bacc("TRN2", target_bir_lowering=False, debug=False,
                   num_devices=NCORES, num_swdge_queues=4)

    # ---- DRAM tensors ----
    def din(name, shape, d):
        return nc.dram_tensor(name, shape, d, kind="ExternalInput")

    naT = din("naT", [2, NP_], dt.float16)
    gidx = din("gidx", [128, 2 * nstream_h // 16], dt.int16)
    ohc = din("ohc", [nch // 8, 128, 8, F], dt.float8e4)
    ohr = din("ohr", [nch2 // 8, 128, 8, F], dt.float8e4)
    eav = din("eav", [128, nch2], dt.float16)
    cip = din("cip", [2, NP_], dt.float32)
    W0 = din("W0", [2, F], dt.float16)
    Wn = [din(f"Wn{i}", [F, F], dt.float16) for i in range(L)]
    Wb = [din(f"Wb{i}", [F, F], dt.float16) for i in range(L)]
    W1 = [din(f"W1{i}", [F, F], dt.float16) for i in range(L)]
    W2 = [din(f"W2{i}", [F, F], dt.float16) for i in range(L)]
    # ea outer lhsT rows: [Wedge_i ; bedge_i] fp32 [2, F]; aggr bias outer [1,F]
    Wec = [din(f"Wec{i}", [2, F], dt.float32) for i in range(L)]
    Bnb = [din(f"Bnb{i}", [1, F], dt.float32) for i in range(L)]
    # BN affine params as [128,1] fp32 columns: (g, bt) per bn
    gcol = {}
    for nm in ("g0", "bt0"):
        gcol[nm] = din(nm, [F, 1], dt.float32)
    for i in range(L):
        for nm in ("gn", "btn", "ge", "bte", "gnb", "btnb",
                   "gm1", "btm1", "gm2", "btm2"):
            gcol[f"{nm}{i}"] = din(f"{nm}{i}", [F, 1], dt.float32)

    out = nc.dram_tensor("out", [F, NP_], dt.float32, kind="ExternalOutput")

    with tile.TileContext(nc) as tc:
        import contextlib
        ctx = contextlib.ExitStack()
        with ctx:
            sb = ctx.enter_context(tc.tile_pool(name="sb", bufs=1))
            wpool = ctx.enter_context(tc.tile_pool(name="wp", bufs=1))
            tp = ctx.enter_context(tc.tile_pool(name="tp", bufs=2))
            gp = ctx.enter_context(tc.tile_pool(name="gp", bufs=4))
            ohp = ctx.enter_context(tc.tile_pool(name="ohp", bufs=3))
            ohrp = ctx.enter_context(tc.tile_pool(name="ohrp", bufs=2))
            hst = ctx.enter_context(tc.tile_pool(name="hst", bufs=3))
            stp = ctx.enter_context(tc.tile_pool(name="stp", bufs=1))
            afp = ctx.enter_context(tc.tile_pool(name="afp", bufs=6))
            ps_mm = ctx.enter_context(tc.tile_pool(name="psmm", bufs=2, space="PSUM"))
            ps_sc = ctx.enter_context(tc.tile_pool(name="pssc", bufs=2, space="PSUM"))
            ps_ou = ctx.enter_context(tc.tile_pool(name="psou", bufs=1, space="PSUM"))
            ps_sp = ctx.enter_context(tc.tile_pool(name="pssp", bufs=2, space="PSUM"))
            dram = ctx.enter_context(tc.tile_pool(name="dram", bufs=1, space="DRAM"))

            # persistent SBUF
            xbuf = sb.tile([F, NP_], dt.float16)       # x / xn / y1 / m1 ... in-place
            gidx_sb = sb.tile([128, 2 * nstream_h // 16], dt.int16)
            nc.sync.dma_start(gidx_sb[:], gidx.ap())
            eav_sb = sb.tile([128, nch2], dt.float16)
            nc.sync.dma_start(eav_sb[:], eav.ap())
            aggr_sh = sb.tile([F, SHARD], dt.float32)
            s_sh = sb.tile([1, SHARD], dt.float32)

            # weights resident
            W0_sb = wpool.tile([2, F], dt.float16)
            nc.sync.dma_start(W0_sb[:], W0.ap())
            Wn_sb, Wb_sb, W1_sb, W2_sb, Wec_sb, Bnb_sb = [], [], [], [], [], []
            for i in range(L):
                for lst, t_ in ((Wn_sb, Wn[i]), (Wb_sb, Wb[i]),
                                (W1_sb, W1[i]), (W2_sb, W2[i])):
                    w = wpool.tile([F, F], dt.float16, tag=f"w{len(lst)}_{t_.name}")
                    nc.sync.dma_start(w[:], t_.ap())
                    lst.append(w)
                w = wpool.tile([2, F], dt.float32, tag=f"wec{i}")
                nc.sync.dma_start(w[:], Wec[i].ap())
                Wec_sb.append(w)
                w = wpool.tile([1, F], dt.float32, tag=f"bnb{i}")
                nc.sync.dma_start(w[:], Bnb[i].ap())
                Bnb_sb.append(w)
            gc_sb = {}
            for nm, t_ in gcol.items():
                w = wpool.tile([F, 1], dt.float32, tag=f"p{nm}")
                nc.sync.dma_start(w[:], t_.ap())
                gc_sb[nm] = w

            # DRAM scratch
            htab = dram.tile([NP_, F], dt.float16)
            ag_ins = [dram.tile([F, SHARD], dt.float16, tag=f"agi{i}", name=f"agi{i}")
                      for i in range(L)]
            ag_outs = [dram.tile([NCORES, F, SHARD], dt.float16,
                                 addr_space="Shared", tag=f"ago{i}", name=f"ago{i}")
                       for i in range(L)]
            s_in = dram.tile([1, SHARD], dt.float32)
            s_out = dram.tile([NCORES, 1, SHARD], dt.float32, addr_space="Shared")

            # ---------- helpers ----------
            def finalize_bn(g, bt, ssum, ssq):
                """column affine: returns (scale, shift) [128,1] fp32"""
                mean = afp.tile([F, 1], dt.float32)
                nc.vector.tensor_scalar_mul(mean[:], ssum[:], 1.0 / NREAL)
                # var = ssq/N - mean^2
                m2t = afp.tile([F, 1], dt.float32)
                nc.scalar.activation(m2t[:], mean[:], ACTF.Square)
                var = afp.tile([F, 1], dt.float32)
                nc.vector.scalar_tensor_tensor(
                    out=var[:], in0=ssq[:], scalar=1.0 / NREAL, in1=m2t[:],
                    op0=OP.mult, op1=OP.subtract)
                nc.vector.tensor_scalar_add(var[:], var[:], EPS)
                lnv = afp.tile([F, 1], dt.float32)
                nc.scalar.activation(lnv[:], var[:], ACTF.Ln)
                isig = afp.tile([F, 1], dt.float32)
                nc.scalar.activation(isig[:], lnv[:], ACTF.Exp, scale=-0.5)
                scale = afp.tile([F, 1], dt.float32)
                nc.vector.tensor_mul(scale[:], g[:], isig[:])
                nscale = afp.tile([F, 1], dt.float32)
                nc.vector.tensor_scalar_mul(nscale[:], scale[:], -1.0)
                shift = afp.tile([F, 1], dt.float32)
                nc.vector.scalar_tensor_tensor(
                    out=shift[:], in0=mean[:], scalar=nscale[:], in1=bt[:],
                    op0=OP.mult, op1=OP.add)
                return scale, shift

            def stat_pass_psum(psum, c, ssum_sl, ssq_sl, trash):
                w = CHUNK if (c + 1) * CHUNK <= NREAL else max(0, NREAL - c * CHUNK)
                if w == 0:
                    return
                nc.vector.tensor_reduce(ssum_sl[:, c:c + 1], psum[:, :w], AX, OP.add)
                nc.scalar.activation(trash[:, :w], psum[:, :w], ACTF.Square,
                                     accum_out=ssq_sl[:, c:c + 1])

            def slot_reduce(slots):
                r = afp.tile([F, 1], dt.float32)
                nc.vector.tensor_reduce(r[:], slots[:], AX, OP.add)
                return r

            # ---------- layer 0: x0 = relu(bn0(naT @ W0)) ----------
            ss0 = stp.tile([F, NCHD], dt.float32, tag="ss0")
            nc.vector.memset(ss0[:], 0.0)
            sq0 = stp.tile([F, NCHD], dt.float32, tag="sq0")
            nc.vector.memset(sq0[:], 0.0)
            for c in range(NCHD):
                nat = tp.tile([2, CHUNK], dt.float16, tag="nat")
                nc.sync.dma_start(nat[:], naT.ap()[:, c * CHUNK:(c + 1) * CHUNK])
                ps = ps_mm.tile([F, CHUNK], dt.float32, tag="mm")
                nc.tensor.matmul(ps[:], lhsT=W0_sb[:], rhs=nat[:], start=True, stop=True)
                tr = tp.tile([F, CHUNK], dt.float32, tag="tr")
                stat_pass_psum(ps, c, ss0, sq0, tr)
                nc.scalar.activation(xbuf[:, c * CHUNK:(c + 1) * CHUNK], ps[:],
                                     ACTF.Copy)
            sc0, sh0 = finalize_bn(gc_sb["g0"], gc_sb["bt0"], slot_reduce(ss0), slot_reduce(sq0))
            for c in range(NCHD):
                sl = xbuf[:, c * CHUNK:(c + 1) * CHUNK]
                nc.scalar.activation(sl, sl, ACTF.Relu, bias=sh0[:], scale=sc0[:])

            # ---------- s pass: s = segment_sum(edge_attr, row) + AllGather ----------
            for b in range(NBLK):
                pss = ps_sp.tile([1, F], dt.float32, tag="sp")
                for k in range(cpb2):
                    ci = b * cpb2 + k
                    g8 = ci // 8
                    if ci % 8 == 0:
                        ohrt = ohrp.tile([128, 8, F], dt.float8e4, tag="ohr")
                        nc.sync.dma_start(ohrt[:], ohr.ap()[g8])
                    nc.tensor.matmul(pss[:], lhsT=eav_sb[:, ci:ci + 1],
                                     rhs=ohrt[:, ci % 8, :],
                                     start=(k == 0), stop=(k == cpb2 - 1))
                nc.vector.tensor_copy(s_sh[:, b * F:(b + 1) * F], pss[:])
            nc.gpsimd.dma_start(s_in[:], s_sh[:])
            nc.gpsimd.collective_compute(
                "AllGather", OP.bypass, replica_groups=[list(range(NCORES))],
                ins=[s_in.opt()], outs=[s_out.opt()])

            # ---------- layers ----------
            for i in range(L):
                ag_in = ag_ins[i]
                ag_out = ag_outs[i]
                # h-pass: node-major h = x @ Wb[i] -> htab
                for b in range(NP_ // F):
                    ps = ps_sc.tile([F, F], dt.float32, tag="sc")
                    nc.tensor.matmul(ps[:], lhsT=xbuf[:, b * F:(b + 1) * F],
                                     rhs=Wb_sb[i][:], start=True, stop=True)
                    hs = hst.tile([F, F], dt.float16, tag="hs")
                    nc.scalar.activation(hs[:], ps[:], ACTF.Copy)
                    nc.sync.dma_start(htab[b * F:(b + 1) * F, :], hs[:])

                # scatter: per half, gather calls + one-hot matmuls
                for h in range(2):
                    base = h * nstream_h
                    for call in range(ncalls_h):
                        gt = gp.tile([128, 8, F], dt.float16, tag="g")
                        j0 = base + call * GCALL
                        nc.gpsimd.dma_gather(
                            out_ap=gt[:],
                            in_ap=htab[h * HALF:(h + 1) * HALF, :],
                            idxs_ap=gidx_sb[:, j0 // 16:(j0 + GCALL) // 16],
                            num_idxs=GCALL, num_idxs_reg=GCALL, elem_size=F,
                            queue_num=call % 4)
                        for k8 in range(8):
                            ci = call * 8 + k8          # chunk within half
                            if ci % 8 == 0:
                                oht = ohp.tile([128, 8, F], dt.float8e4, tag="oh")
                                nc.sync.dma_start(
                                    oht[:], ohc.ap()[(base // F + ci) // 8])
                            b = ci // cpb
                            k = ci % cpb
                            if k == 0:
                                psb = ps_sc.tile([F, F], dt.float32, tag="sc")
                            nc.tensor.matmul(psb[:], lhsT=gt[:, k8, :],
                                             rhs=oht[:, ci % 8, :],
                                             start=(k == 0), stop=(k == cpb - 1))
                            if k == cpb - 1:
                                dst = aggr_sh[:, b * F:(b + 1) * F]
                                if h == 0:
                                    nc.scalar.activation(dst, psb[:], ACTF.Copy)
                                else:
                                    nc.vector.scalar_tensor_tensor(
                                        out=dst, in0=psb[:], scalar=1.0,
                                        in1=dst, op0=OP.mult, op1=OP.add)

                # ship aggr shard
                for j in range(SHARD // CHUNK):
                    agb = tp.tile([F, CHUNK], dt.float16, tag="agb")
                    nc.vector.tensor_copy(agb[:], aggr_sh[:, j * CHUNK:(j + 1) * CHUNK])
                    nc.gpsimd.dma_start(ag_in[:, j * CHUNK:(j + 1) * CHUNK], agb[:])
                nc.gpsimd.collective_compute(
                    "AllGather", OP.bypass, replica_groups=[list(range(NCORES))],
                    ins=[ag_in.opt()], outs=[ag_out.opt()])

                # xn stats pass (values discarded; xn recomputed later)
                ssn = stp.tile([F, NCHD], dt.float32, tag="ssn")
                nc.vector.memset(ssn[:], 0.0)
                sqn = stp.tile([F, NCHD], dt.float32, tag="sqn")
                nc.vector.memset(sqn[:], 0.0)
                for c in range(NCHD):
                    ps = ps_mm.tile([F, CHUNK], dt.float32, tag="mm")
                    nc.tensor.matmul(ps[:], lhsT=Wn_sb[i][:],
                                     rhs=xbuf[:, c * CHUNK:(c + 1) * CHUNK],
                                     start=True, stop=True)
                    tr = tp.tile([F, CHUNK], dt.float32, tag="tr")
                    stat_pass_psum(ps, c, ssn, sqn, tr)
                sc_n, sh_n = finalize_bn(gc_sb[f"gn{i}"], gc_sb[f"btn{i}"], slot_reduce(ssn), slot_reduce(sqn))

                # ea stats pass + aggr stats pass (both from psum/AG stream)
                sse = stp.tile([F, NCHD], dt.float32, tag="sse")
                nc.vector.memset(sse[:], 0.0)
                sqe = stp.tile([F, NCHD], dt.float32, tag="sqe")
                nc.vector.memset(sqe[:], 0.0)
                ssa = stp.tile([F, NCHD], dt.float32, tag="ssa")
                nc.vector.memset(ssa[:], 0.0)
                sqa = stp.tile([F, NCHD], dt.float32, tag="sqa")
                nc.vector.memset(sqa[:], 0.0)
                for c in range(NCHD):
                    scc = tp.tile([2, CHUNK], dt.float32, tag="scc")
                    cint = tp.tile([1, CHUNK], dt.float32, tag="cint")
                    r = c * CHUNK // SHARD
                    off = c * CHUNK % SHARD
                    nc.sync.dma_start(
                        scc[0:1, :],
                        s_out[r, :, off:off + CHUNK])
                    nc.sync.dma_start(scc[1:2, :],
                                      cip.ap()[0:1, c * CHUNK:(c + 1) * CHUNK])
                    nc.sync.dma_start(cint[:],
                                      cip.ap()[1:2, c * CHUNK:(c + 1) * CHUNK])
                    pse = ps_ou.tile([F, CHUNK], dt.float32, tag="pse")
                    nc.tensor.matmul(pse[:], lhsT=Wec_sb[i][:], rhs=scc[0:2, :],
                                     start=True, stop=True)
                    tr = tp.tile([F, CHUNK], dt.float32, tag="tr")
                    stat_pass_psum(pse, c, sse, sqe, tr)
                    # aggr chunk = AG + bnb (x) c_in
                    psa = ps_ou.tile([F, CHUNK], dt.float32, tag="psa")
                    nc.tensor.matmul(psa[:], lhsT=Bnb_sb[i][:], rhs=cint[:],
                                     start=True, stop=True)
                    agt = tp.tile([F, CHUNK], dt.float16, tag="agt")
                    nc.sync.dma_start(agt[:], ag_out[r, :, off:off + CHUNK])
                    agf = tp.tile([F, CHUNK], dt.float32, tag="agf")
                    nc.vector.scalar_tensor_tensor(
                        out=agf[:], in0=agt[:], scalar=1.0, in1=psa[:],
                        op0=OP.mult, op1=OP.add)
                    stat_pass_psum(agf, c, ssa, sqa, tr)
                sc_e, sh_e = finalize_bn(gc_sb[f"ge{i}"], gc_sb[f"bte{i}"], slot_reduce(sse), slot_reduce(sqe))
                sc_a, sh_a = finalize_bn(gc_sb[f"gnb{i}"], gc_sb[f"btnb{i}"], slot_reduce(ssa), slot_reduce(sqa))
                # combined shift for y1 = relu(sum of three bn outputs)
                shsum = afp.tile([F, 1], dt.float32)
                nc.vector.scalar_tensor_tensor(
                    out=shsum[:], in0=sh_n[:], scalar=1.0, in1=sh_e[:],
                    op0=OP.mult, op1=OP.add)
                nc.vector.scalar_tensor_tensor(
                    out=shsum[:], in0=shsum[:], scalar=1.0, in1=sh_a[:],
                    op0=OP.mult, op1=OP.add)

                # y1 pass (in place into xbuf) + m1 matmul + m1 stats
                ss1 = stp.tile([F, NCHD], dt.float32, tag="ss1")
                nc.vector.memset(ss1[:], 0.0)
                sq1 = stp.tile([F, NCHD], dt.float32, tag="sq1")
                nc.vector.memset(sq1[:], 0.0)
                for c in range(NCHD):
                    sl = xbuf[:, c * CHUNK:(c + 1) * CHUNK]
                    ps = ps_mm.tile([F, CHUNK], dt.float32, tag="mm")
                    nc.tensor.matmul(ps[:], lhsT=Wn_sb[i][:], rhs=sl,
                                     start=True, stop=True)
                    scc = tp.tile([2, CHUNK], dt.float32, tag="scc")
                    cint = tp.tile([1, CHUNK], dt.float32, tag="cint")
                    r = c * CHUNK // SHARD
                    off = c * CHUNK % SHARD
                    nc.sync.dma_start(scc[0:1, :], s_out[r, :, off:off + CHUNK])
                    nc.sync.dma_start(scc[1:2, :],
                                      cip.ap()[0:1, c * CHUNK:(c + 1) * CHUNK])
                    nc.sync.dma_start(cint[:],
                                      cip.ap()[1:2, c * CHUNK:(c + 1) * CHUNK])
                    pse = ps_ou.tile([F, CHUNK], dt.float32, tag="pse")
                    nc.tensor.matmul(pse[:], lhsT=Wec_sb[i][:], rhs=scc[0:2, :],
                                     start=True, stop=True)
                    psa = ps_ou.tile([F, CHUNK], dt.float32, tag="psa")
                    nc.tensor.matmul(psa[:], lhsT=Bnb_sb[i][:], rhs=cint[:],
                                     start=True, stop=True)
                    agt = tp.tile([F, CHUNK], dt.float16, tag="agt")
                    nc.sync.dma_start(agt[:], ag_out[r, :, off:off + CHUNK])
                    # u = sc_n*xn ; u += sc_a*(agt+psa) ; u += sc_e*ea ; relu(+shsum)
                    u = tp.tile([F, CHUNK], dt.float32, tag="u")
                    nc.vector.scalar_tensor_tensor(
                        out=u[:], in0=agt[:], scalar=1.0, in1=psa[:],
                        op0=OP.mult, op1=OP.add)
                    # u2 = (u * sc_a) + (xn * sc_n): two stt ops
                    u2 = tp.tile([F, CHUNK], dt.float32, tag="u2")
                    nc.vector.tensor_scalar_mul(u2[:], ps[:], sc_n[:])
                    nc.vector.scalar_tensor_tensor(
                        out=u2[:], in0=u[:], scalar=sc_a[:], in1=u2[:],
                        op0=OP.mult, op1=OP.add)
                    nc.vector.scalar_tensor_tensor(
                        out=u2[:], in0=pse[:], scalar=sc_e[:], in1=u2[:],
                        op0=OP.mult, op1=OP.add)
                    y1 = tp.tile([F, CHUNK], dt.float16, tag="y1")
                    nc.scalar.activation(y1[:], u2[:], ACTF.Relu,
                                         bias=shsum[:], scale=1.0)
                    # m1
                    ps1 = ps_mm.tile([F, CHUNK], dt.float32, tag="mm")
                    nc.tensor.matmul(ps1[:], lhsT=W1_sb[i][:], rhs=y1[:],
                                     start=True, stop=True)
                    tr = tp.tile([F, CHUNK], dt.float32, tag="tr")
                    stat_pass_psum(ps1, c, ss1, sq1, tr)
                    nc.scalar.activation(sl, ps1[:], ACTF.Copy)
                sc1, sh1 = finalize_bn(gc_sb[f"gm1{i}"], gc_sb[f"btm1{i}"], slot_reduce(ss1), slot_reduce(sq1))

                # y2 = relu(bn(m1)) in place ; m2 ; stats
                ss2 = stp.tile([F, NCHD], dt.float32, tag="ss2")
                nc.vector.memset(ss2[:], 0.0)
                sq2 = stp.tile([F, NCHD], dt.float32, tag="sq2")
                nc.vector.memset(sq2[:], 0.0)
                for c in range(NCHD):
                    sl = xbuf[:, c * CHUNK:(c + 1) * CHUNK]
                    y2 = tp.tile([F, CHUNK], dt.float16, tag="y2")
                    nc.scalar.activation(y2[:], sl, ACTF.Relu,
                                         bias=sh1[:], scale=sc1[:])
                    ps2 = ps_mm.tile([F, CHUNK], dt.float32, tag="mm")
                    nc.tensor.matmul(ps2[:], lhsT=W2_sb[i][:], rhs=y2[:],
                                     start=True, stop=True)
                    tr = tp.tile([F, CHUNK], dt.float32, tag="tr")
                    stat_pass_psum(ps2, c, ss2, sq2, tr)
                    nc.scalar.activation(sl, ps2[:], ACTF.Copy)
                sc2, sh2 = finalize_bn(gc_sb[f"gm2{i}"], gc_sb[f"btm2{i}"], slot_reduce(ss2), slot_reduce(sq2))

                # x_next = relu(bn(m2)) in place (+ fp32 out on last layer)
                for c in range(NCHD):
                    sl = xbuf[:, c * CHUNK:(c + 1) * CHUNK]
                    if i == L - 1:
                        of = tp.tile([F, CHUNK], dt.float32, tag="of")
                        nc.scalar.activation(of[:], sl, ACTF.Relu,
                                             bias=sh2[:], scale=sc2[:])
                        nc.sync.dma_start(out.ap()[:, c * CHUNK:(c + 1) * CHUNK],
                                          of[:])
                    else:
                        nc.scalar.activation(sl, sl, ACTF.Relu,
                                             bias=sh2[:], scale=sc2[:])

    nc.compile()
    return nc


def kernel(**inputs):
    import sys
    for p in ("/opt/trn_rl_repo",):
        if p not in sys.path:
            sys.path.insert(0, p)
    from concourse import bass_utils

    meta = _prep(inputs["node_attr"], inputs["edge_index"], inputs["edge_attr"])

    nc = _build(meta, inputs)

    def col(v):
        return np.ascontiguousarray(v.astype(np.float32).reshape(F, 1))

    base = dict(
        naT=meta["naT"], cip=meta["ci_pack"].astype(np.float32),
        W0=inputs["W0"].astype(BF16),
        g0=col(inputs["g0"]), bt0=col(inputs["bt0"]),
    )
    for i in range(L):
        base[f"Wn{i}"] = inputs["Wnode"][i].astype(BF16)
        base[f"Wb{i}"] = inputs["Wnb"][i].astype(BF16)
        base[f"W1{i}"] = inputs["Wm1"][i].astype(BF16)
        base[f"W2{i}"] = inputs["Wm2"][i].astype(BF16)
        base[f"Wec{i}"] = np.ascontiguousarray(
            np.stack([inputs["Wedge"][i][0], inputs["bedge"][i]]).astype(np.float32))
        base[f"Bnb{i}"] = np.ascontiguousarray(
            inputs["bnb"][i].astype(np.float32).reshape(1, F))
        for nm in ("gn", "btn", "ge", "bte", "gnb", "btnb",
                   "gm1", "btm1", "gm2", "btm2"):
            base[f"{nm}{i}"] = col(inputs[nm][i])

    in_maps = []
    for r in range(NCORES):
        m = dict(base)
        m["gidx"] = meta["gidx"][r]
        m["ohc"] = meta["ohc"][r]
        m["ohr"] = meta["ohr"][r]
        m["eav"] = meta["eav_t"][r]
        in_maps.append(m)

    res = bass_utils.run_bass_kernel_spmd(
        nc, in_maps, core_ids=list(range(NCORES)))
    xT = res.results[0]["out"]
    return np.ascontiguousarray(xT.T[:NREAL]).astype(np.float32)


if __name__ == "__main__":
    pass

